# revision 1
# baseline (speedup 1.0000x reference)
"""Trainium2 kernel for nn_DoubleAffineNet.

Math: the module's output is phi + psi - I where phi, psi are 3x3 affine
matrices built from pooled image statistics. phi needs mean(x), mean(y).
psi needs mean(x) and mean(y_comp), where y_comp is y bilinearly warped by
the near-identity affine map phi^{-1}.

Key identity: only the MEAN of y_comp is needed. Writing the warp-mean as
sum_{p,q} Y[p,q] * G[p,q] (G = bilinear splat weights of the affinely
mapped output lattice), a partition-of-unity argument shows that for
sub-pixel displacement fields (|u|,|v| < 0.5, which holds for this
problem's near-identity maps; asserted at runtime on the host), G is the
constant kappa = (1-a')(1-d') + b*c everywhere except the four border
rows/cols. Hence

    sum(y_comp) = kappa * sum(y) + sum_border Y*(G_true - kappa)

The device kernel therefore only computes the memory-bound statistics:
per-sample sum(x), sum(y), and the four border strips of y. The remaining
O(B*(3x3 + 4*1024)) algebra runs on the host (f32 where the reference is
f32, f64 for the border correction).

Sharding: pure data parallel, one sample per NeuronCore (B=8, 8 cores).
"""

import numpy as np

H = 1024
W = 1024
NCHUNK = 8  # row-blocks of 128 rows per image
OUT_LEN = 2 + 4 * 1024  # [sum_x, sum_y, row0, row1023, col0, col1023]

_CACHE = {}


def _build_program():
    """Build the per-core Bass program: sums of x and y + y border strips."""
    import concourse.bass as bass
    import concourse.bacc as bacc
    import concourse.tile as tile
    from concourse import mybir

    f32 = mybir.dt.float32
    nc = bacc.Bacc("TRN2", target_bir_lowering=False, debug=False, num_devices=8)

    xd = nc.dram_tensor("x", [H, W], f32, kind="ExternalInput").ap()
    yd = nc.dram_tensor("y", [H, W], f32, kind="ExternalInput").ap()
    outd = nc.dram_tensor("out", [OUT_LEN], f32, kind="ExternalOutput").ap()

    with tile.TileContext(nc) as tc:
        with (
            tc.tile_pool(name="chunks", bufs=6) as chunks,
            tc.tile_pool(name="acc", bufs=1) as acc,
            tc.tile_pool(name="small", bufs=1) as small,
        ):
            partials = acc.tile([128, 2 * NCHUNK], f32)
            col0 = acc.tile([128, NCHUNK], f32)
            col1 = acc.tile([128, NCHUNK], f32)

            for img_idx, src in enumerate((xd, yd)):
                for c in range(NCHUNK):
                    t = chunks.tile([128, W], f32, tag="chunk")
                    nc.sync.dma_start(out=t[:], in_=src[c * 128:(c + 1) * 128, :])
                    nc.vector.tensor_reduce(
                        out=partials[:, img_idx * NCHUNK + c : img_idx * NCHUNK + c + 1],
                        in_=t[:],
                        axis=mybir.AxisListType.X,
                        op=mybir.AluOpType.add,
                    )
                    if img_idx == 1:
                        nc.vector.tensor_copy(col0[:, c : c + 1], t[:, 0:1])
                        nc.vector.tensor_copy(col1[:, c : c + 1], t[:, W - 1 : W])
                        if c == 0:
                            nc.sync.dma_start(out=outd[2 : 2 + W], in_=t[0:1, :])
                        if c == NCHUNK - 1:
                            nc.sync.dma_start(
                                out=outd[2 + W : 2 + 2 * W], in_=t[127:128, :]
                            )

            # cross-partition reduction of the 16 per-partition partials via PE
            ones = small.tile([128, 1], f32)
            nc.vector.memset(ones[:], 1.0)
            with tc.tile_pool(name="psum", bufs=1, space="PSUM") as psum_pool:
                ps = psum_pool.tile([1, 2 * NCHUNK], f32)
                nc.tensor.matmul(ps[:], ones[:], partials[:])
                red = small.tile([1, 2 * NCHUNK], f32)
                nc.scalar.copy(red[:], ps[:])
            sums = small.tile([1, 2], f32)
            nc.vector.tensor_reduce(
                out=sums[:],
                in_=red[:].rearrange("p (i c) -> p i c", i=2),
                axis=mybir.AxisListType.X,
                op=mybir.AluOpType.add,
            )
            nc.sync.dma_start(out=outd[0:2], in_=sums[:])

            # column strips: DRAM index 2 + 2W + (c*128 + p) <- col0[p, c]
            nc.sync.dma_start(
                out=outd[2 + 2 * W : 2 + 3 * W].rearrange("(c p) -> p c", p=128),
                in_=col0[:],
            )
            nc.sync.dma_start(
                out=outd[2 + 3 * W : 2 + 4 * W].rearrange("(c p) -> p c", p=128),
                in_=col1[:],
            )

    nc.compile()
    return nc


def _get_program():
    if "nc" not in _CACHE:
        _CACHE["nc"] = _build_program()
    return _CACHE["nc"]


def _tent(z):
    return np.maximum(0.0, 1.0 - np.abs(z))


def _warp_sum(sum_y, row0, row1, c0, c1, A):
    """sum(y_comp) from sum(y) + border strips, given phi_inv = A (f32)."""
    A64 = A.astype(np.float64)
    ap, bb = A64[0, 0] - 1.0, A64[0, 1]
    cc, dp = A64[1, 0], A64[1, 1] - 1.0
    e1, e2 = 1023.0 * A64[0, 2], 1023.0 * A64[1, 2]

    # sub-pixel displacement assumption |u|,|v| < 0.5 (checked at corners;
    # the fields are affine so corners bound the interior)
    mu = max(abs(ap * i + bb * j + e1) for i in (0.0, 1023.0) for j in (0.0, 1023.0))
    mv = max(abs(cc * i + dp * j + e2) for i in (0.0, 1023.0) for j in (0.0, 1023.0))
    assert mu < 0.5 and mv < 0.5, (mu, mv)

    kappa = (1.0 - ap) * (1.0 - dp) + bb * cc

    def g_true(p, q):
        g = np.zeros(np.broadcast(p, q).shape)
        for di in (-1, 0, 1):
            for dj in (-1, 0, 1):
                i_, j_ = p - di, q - dj
                valid = (i_ >= 0) & (i_ < H) & (j_ >= 0) & (j_ < W)
                z1 = ap * i_ + bb * j_ + e1 - di
                z2 = cc * i_ + dp * j_ + e2 - dj
                g += _tent(z1) * _tent(z2) * valid
        return g

    qs = np.arange(W, dtype=np.float64)
    ps = np.arange(1, H - 1, dtype=np.float64)
    ds = 0.0
    ds += np.sum(row0.astype(np.float64) * (g_true(0.0, qs) - kappa))
    ds += np.sum(row1.astype(np.float64) * (g_true(1023.0, qs) - kappa))
    ds += np.sum(c0[1:-1].astype(np.float64) * (g_true(ps, 0.0) - kappa))
    ds += np.sum(c1[1:-1].astype(np.float64) * (g_true(ps, 1023.0) - kappa))

    return kappa * float(sum_y) + ds


def _affine_f32(feat32, Wl, bl):
    M = (feat32 @ Wl + bl).reshape(3, 3)
    return np.eye(3, dtype=np.float32) + np.float32(0.01) * M


def kernel(x, y, Wpsi, bpsi, Wphi, bphi):
    from concourse import bass2jax

    B = x.shape[0]
    assert x.shape == (B, 1, H, W) and y.shape == (B, 1, H, W)

    nc = _get_program()
    in_maps = [
        {"x": np.ascontiguousarray(x[b, 0]), "y": np.ascontiguousarray(y[b, 0])}
        for b in range(B)
    ]
    results = bass2jax.run_bass_via_pjrt(nc, in_maps, n_cores=B)

    out = np.empty((B, 3, 3), dtype=np.float32)
    inv_hw = 1.0 / float(H * W)
    for b in range(B):
        r = np.asarray(results[b]["out"], dtype=np.float32).reshape(-1)
        sum_x, sum_y = float(r[0]), float(r[1])
        row0 = r[2 : 2 + W]
        row1 = r[2 + W : 2 + 2 * W]
        c0 = r[2 + 2 * W : 2 + 3 * W]
        c1 = r[2 + 3 * W : 2 + 4 * W]

        mean_x = np.float32(sum_x * inv_hw)
        mean_y = np.float32(sum_y * inv_hw)
        phi = _affine_f32(np.array([mean_x, mean_y], np.float32), Wpsi, bpsi)
        A = np.linalg.inv(phi)

        s_comp = _warp_sum(sum_y, row0, row1, c0, c1, A)
        mean_yc = np.float32(s_comp * inv_hw)

        psi = _affine_f32(np.array([mean_x, mean_yc], np.float32), Wphi, bphi)
        out[b] = phi + psi - np.eye(3, dtype=np.float32)
    return out


# revision 11
# speedup vs baseline: 1.3814x; 1.3814x over previous
"""Trainium2 kernel for nn_DoubleAffineNet.

Math: the module's output is phi + psi - I where phi, psi are 3x3 affine
matrices built from pooled image statistics. phi needs mean(x), mean(y).
psi needs mean(x) and mean(y_comp), where y_comp is y bilinearly warped by
the near-identity affine map phi^{-1}.

Key identity: only the MEAN of y_comp is needed. Writing the warp-mean as
sum_{p,q} Y[p,q] * G[p,q] (G = bilinear splat weights of the affinely
mapped output lattice), a partition-of-unity argument shows that for
sub-pixel displacement fields (|u|,|v| < 0.5, which holds for this
problem's near-identity maps; asserted at runtime on the host), G is the
constant kappa = (1-a')(1-d') + b*c everywhere except the four border
rows/cols. Hence

    sum(y_comp) = kappa * sum(y) + sum_border Y*(G_true - kappa)

The device kernel therefore only computes the memory-bound statistics:
per-sample sum(x), sum(y), and the four border strips of y. The remaining
O(B*(3x3 + 4*1024)) algebra runs on the host (f32 where the reference is
f32, f64 for the border correction).

Sharding: pure data parallel, one sample per NeuronCore (B=8, 8 cores).
"""

import numpy as np

H = 1024
W = 1024
NCHUNK = 4  # [128, 2048] chunks (256 rows) per image
OUT_LEN = 5 * 1024  # [partials(1024), row0, row1023, col0, col1023]

_CACHE = {}


def _build_program():
    """Per-core raw Bass program (no TileContext: avoids the expensive
    end-of-kernel drain/barrier butterfly).

    Streams x then y as 8 chunk DMAs of [128, 2048] (partition p of chunk c
    holds rows c*256 + {p, 128+p}).  Vector reduces each chunk to a
    [128, 1] partial; Scalar extracts the y border strips.  Outputs the
    [128, 8] partials + 4 strips; the final tiny reduction happens on host.
    """
    import concourse.bacc as bacc
    from concourse import mybir

    f32 = mybir.dt.float32
    nc = bacc.Bacc("TRN2", target_bir_lowering=False, debug=False, num_devices=8)

    xd = nc.dram_tensor("x", [H, W], f32, kind="ExternalInput").ap()
    yd = nc.dram_tensor("y", [H, W], f32, kind="ExternalInput").ap()
    outd = nc.dram_tensor("out", [OUT_LEN], f32, kind="ExternalOutput").ap()

    import contextlib

    with contextlib.ExitStack() as ctx:
        bufs = [
            ctx.enter_context(nc.sbuf_tensor(f"chunk{k}", [128, 2 * W], f32))
            for k in range(2 * NCHUNK)
        ]
        partials = ctx.enter_context(nc.sbuf_tensor("partials", [128, 8], f32))
        col0 = ctx.enter_context(nc.sbuf_tensor("col0", [128, 8], f32))
        col1 = ctx.enter_context(nc.sbuf_tensor("col1", [128, 8], f32))
        dma_in = ctx.enter_context(nc.semaphore("dma_in"))
        ved = ctx.enter_context(nc.semaphore("ved"))
        sed = ctx.enter_context(nc.semaphore("sed"))
        dma_out = ctx.enter_context(nc.semaphore("dma_out"))
        block = ctx.enter_context(nc.Block())

        def src_chunk(k):
            src = xd if k < NCHUNK else yd
            c = k % NCHUNK
            return src[c * 256:(c + 1) * 256, :].rearrange(
                "(a p) q -> p a q", a=2
            )

        @block.sync
        def _(sync):
            for k in range(2 * NCHUNK):
                dst = bufs[k].ap().rearrange("p (a q) -> p a q", a=2)
                sync.dma_start(out=dst, in_=src_chunk(k)).then_inc(
                    dma_in, 16
                )
            # row strips straight from the resident y chunks (never recycled)
            sync.wait_ge(dma_in, 16 * 2 * NCHUNK)
            sync.dma_start(
                out=outd[1024:2048].rearrange("(p q) -> p q", p=1),
                in_=bufs[NCHUNK][0:1, 0:W],
            ).then_inc(dma_out, 16)
            sync.dma_start(
                out=outd[2048:3072].rearrange("(p q) -> p q", p=1),
                in_=bufs[2 * NCHUNK - 1][127:128, W : 2 * W],
            ).then_inc(dma_out, 16)
            sync.wait_ge(ved, 2 * NCHUNK)
            sync.dma_start(
                out=outd[0:1024].rearrange("(p k) -> p k", k=8),
                in_=partials[:],
            ).then_inc(dma_out, 16)
            sync.wait_ge(sed, 8)
            sync.dma_start(
                out=outd[3072:4096].rearrange("(p k) -> p k", k=8),
                in_=col0[:],
            ).then_inc(dma_out, 16)
            sync.dma_start(
                out=outd[4096:5120].rearrange("(p k) -> p k", k=8),
                in_=col1[:],
            ).then_inc(dma_out, 16)
            sync.wait_ge(dma_out, 80)

        @block.vector
        def _(vector):
            for k in range(2 * NCHUNK):
                vector.wait_ge(dma_in, 16 * (k + 1))
                nc.vector.tensor_reduce(
                    out=partials[:, k : k + 1],
                    in_=bufs[k][:],
                    axis=mybir.AxisListType.X,
                    op=mybir.AluOpType.add,
                ).then_inc(ved, 1)

        @block.scalar
        def _(scalar):
            for j in range(NCHUNK):
                k = NCHUNK + j
                scalar.wait_ge(dma_in, 16 * (k + 1))
                t3 = bufs[k].ap().rearrange("p (a q) -> p a q", a=2)
                nc.scalar.copy(col0[:, 2 * j : 2 * j + 2], t3[:, :, 0]).then_inc(
                    sed, 1
                )
                nc.scalar.copy(
                    col1[:, 2 * j : 2 * j + 2], t3[:, :, W - 1]
                ).then_inc(sed, 1)

    nc.compile()
    return nc


def _get_program():
    if "nc" not in _CACHE:
        _CACHE["nc"] = _build_program()
    return _CACHE["nc"]


def _tent(z):
    return np.maximum(0.0, 1.0 - np.abs(z))


def _warp_sum(sum_y, row0, row1, c0, c1, A):
    """sum(y_comp) from sum(y) + border strips, given phi_inv = A (f32)."""
    A64 = A.astype(np.float64)
    ap, bb = A64[0, 0] - 1.0, A64[0, 1]
    cc, dp = A64[1, 0], A64[1, 1] - 1.0
    e1, e2 = 1023.0 * A64[0, 2], 1023.0 * A64[1, 2]

    # sub-pixel displacement assumption |u|,|v| < 0.5 (checked at corners;
    # the fields are affine so corners bound the interior)
    mu = max(abs(ap * i + bb * j + e1) for i in (0.0, 1023.0) for j in (0.0, 1023.0))
    mv = max(abs(cc * i + dp * j + e2) for i in (0.0, 1023.0) for j in (0.0, 1023.0))
    assert mu < 0.5 and mv < 0.5, (mu, mv)

    kappa = (1.0 - ap) * (1.0 - dp) + bb * cc

    def g_true(p, q):
        g = np.zeros(np.broadcast(p, q).shape)
        for di in (-1, 0, 1):
            for dj in (-1, 0, 1):
                i_, j_ = p - di, q - dj
                valid = (i_ >= 0) & (i_ < H) & (j_ >= 0) & (j_ < W)
                z1 = ap * i_ + bb * j_ + e1 - di
                z2 = cc * i_ + dp * j_ + e2 - dj
                g += _tent(z1) * _tent(z2) * valid
        return g

    qs = np.arange(W, dtype=np.float64)
    ps = np.arange(1, H - 1, dtype=np.float64)
    ds = 0.0
    ds += np.sum(row0.astype(np.float64) * (g_true(0.0, qs) - kappa))
    ds += np.sum(row1.astype(np.float64) * (g_true(1023.0, qs) - kappa))
    ds += np.sum(c0[1:-1].astype(np.float64) * (g_true(ps, 0.0) - kappa))
    ds += np.sum(c1[1:-1].astype(np.float64) * (g_true(ps, 1023.0) - kappa))

    return kappa * float(sum_y) + ds


def _affine_f32(feat32, Wl, bl):
    M = (feat32 @ Wl + bl).reshape(3, 3)
    return np.eye(3, dtype=np.float32) + np.float32(0.01) * M


def kernel(x, y, Wpsi, bpsi, Wphi, bphi):
    from concourse import bass2jax

    B = x.shape[0]
    assert x.shape == (B, 1, H, W) and y.shape == (B, 1, H, W)

    nc = _get_program()
    in_maps = [
        {"x": np.ascontiguousarray(x[b, 0]), "y": np.ascontiguousarray(y[b, 0])}
        for b in range(B)
    ]
    results = bass2jax.run_bass_via_pjrt(nc, in_maps, n_cores=B)

    out = np.empty((B, 3, 3), dtype=np.float32)
    inv_hw = 1.0 / float(H * W)
    for b in range(B):
        r = np.asarray(results[b]["out"], dtype=np.float32).reshape(-1)
        partials = r[0:1024].reshape(128, 8).astype(np.float64)
        sum_x = float(partials[:, 0:4].sum())
        sum_y = float(partials[:, 4:8].sum())
        row0 = r[1024 : 1024 + W]
        row1 = r[1024 + W : 1024 + 2 * W]
        # col strips land p-major: arr[p*8 + k] = y[k*128 + p, col]
        c0 = r[3072 : 3072 + W].reshape(128, 8).T.ravel()
        c1 = r[3072 + W : 3072 + 2 * W].reshape(128, 8).T.ravel()

        mean_x = np.float32(sum_x * inv_hw)
        mean_y = np.float32(sum_y * inv_hw)
        phi = _affine_f32(np.array([mean_x, mean_y], np.float32), Wpsi, bpsi)
        A = np.linalg.inv(phi)

        s_comp = _warp_sum(sum_y, row0, row1, c0, c1, A)
        mean_yc = np.float32(s_comp * inv_hw)

        psi = _affine_f32(np.array([mean_x, mean_yc], np.float32), Wphi, bphi)
        out[b] = phi + psi - np.eye(3, dtype=np.float32)
    return out


# revision 12
# speedup vs baseline: 1.4508x; 1.0502x over previous
"""Trainium2 kernel for nn_DoubleAffineNet.

Math: the module's output is phi + psi - I where phi, psi are 3x3 affine
matrices built from pooled image statistics. phi needs mean(x), mean(y).
psi needs mean(x) and mean(y_comp), where y_comp is y bilinearly warped by
the near-identity affine map phi^{-1}.

Key identity: only the MEAN of y_comp is needed. Writing the warp-mean as
sum_{p,q} Y[p,q] * G[p,q] (G = bilinear splat weights of the affinely
mapped output lattice), a partition-of-unity argument shows that for
sub-pixel displacement fields (|u|,|v| < 0.5, which holds for this
problem's near-identity maps; asserted at runtime on the host), G is the
constant kappa = (1-a')(1-d') + b*c everywhere except the four border
rows/cols. Hence

    sum(y_comp) = kappa * sum(y) + sum_border Y*(G_true - kappa)

The device kernel therefore only computes the memory-bound statistics:
per-sample sum(x), sum(y), and the four border strips of y. The remaining
O(B*(3x3 + 4*1024)) algebra runs on the host (f32 where the reference is
f32, f64 for the border correction).

Sharding: pure data parallel, one sample per NeuronCore (B=8, 8 cores).

Device program (raw bacc, no TileContext — avoids its expensive
end-of-kernel drain/barrier):
  - 8 input DMAs of [128, 2048] (chunk c holds rows c*256 + {p, 128+p})
  - Vector reduces the 4 x-chunks (+ half of the last y-chunk);
    Scalar reduces the y-chunks via ACT accum_out and extracts strips
  - everything lands in one packed [128, 25] "smalls" tile + 2 row strips
  - host does the final ~KB of reduction/algebra in float64
"""

import numpy as np

H = 1024
W = 1024
OUT_LEN = 5248

_CACHE = {}


def _build_program():
    import contextlib

    import concourse.bacc as bacc
    from concourse import mybir

    f32 = mybir.dt.float32
    Copy = mybir.ActivationFunctionType.Copy
    nc = bacc.Bacc("TRN2", target_bir_lowering=False, debug=False, num_devices=8)

    xd = nc.dram_tensor("x", [H, W], f32, kind="ExternalInput").ap()
    yd = nc.dram_tensor("y", [H, W], f32, kind="ExternalInput").ap()
    outd = nc.dram_tensor("out", [OUT_LEN], f32, kind="ExternalOutput").ap()

    with contextlib.ExitStack() as ctx:
        bufs = [
            ctx.enter_context(nc.sbuf_tensor(f"chunk{k}", [128, 2 * W], f32))
            for k in range(8)
        ]
        # smalls cols: 0..3 x-partials, 4..8 y-partials, 9..16 col0, 17..24 col1023
        smalls = ctx.enter_context(nc.sbuf_tensor("smalls", [128, 25], f32))
        scratch = ctx.enter_context(nc.sbuf_tensor("scratch", [128, 2 * W], f32))
        dma_in = ctx.enter_context(nc.semaphore("dma_in"))
        ved = ctx.enter_context(nc.semaphore("ved"))
        sed = ctx.enter_context(nc.semaphore("sed"))
        dma_out = ctx.enter_context(nc.semaphore("dma_out"))
        block = ctx.enter_context(nc.Block())

        def src_chunk(k):
            src = xd if k < 4 else yd
            c = k % 4
            return src[c * 256:(c + 1) * 256, :].rearrange("(a p) q -> p a q", a=2)

        @block.sync
        def _(sync):
            for k in range(8):
                dst = bufs[k].ap().rearrange("p (a q) -> p a q", a=2)
                sync.dma_start(out=dst, in_=src_chunk(k)).then_inc(dma_in, 16)
            # row strips straight from the resident y chunks
            sync.wait_ge(dma_in, 16 * 8)
            sync.dma_start(
                out=outd[3200:4224].rearrange("(p q) -> p q", p=1),
                in_=bufs[4][0:1, 0:W],
            ).then_inc(dma_out, 16)
            sync.dma_start(
                out=outd[4224:5248].rearrange("(p q) -> p q", p=1),
                in_=bufs[7][127:128, W : 2 * W],
            ).then_inc(dma_out, 16)
            sync.wait_ge(ved, 5)
            sync.wait_ge(sed, 12)
            sync.dma_start(
                out=outd[0:3200].rearrange("(p c) -> p c", c=25),
                in_=smalls[:],
            ).then_inc(dma_out, 16)
            sync.wait_ge(dma_out, 48)

        @block.vector
        def _(vector):
            # x chunks
            for k in range(4):
                vector.wait_ge(dma_in, 16 * (k + 1))
                nc.vector.tensor_reduce(
                    out=smalls[:, k : k + 1],
                    in_=bufs[k][:],
                    axis=mybir.AxisListType.X,
                    op=mybir.AluOpType.add,
                ).then_inc(ved, 1)
            # first half of the last y chunk (splits the tail reduce)
            vector.wait_ge(dma_in, 16 * 8)
            nc.vector.tensor_reduce(
                out=smalls[:, 7:8],
                in_=bufs[7][:, 0:W],
                axis=mybir.AxisListType.X,
                op=mybir.AluOpType.add,
            ).then_inc(ved, 1)

        @block.scalar
        def _(scalar):
            for j in range(4):
                k = 4 + j
                scalar.wait_ge(dma_in, 16 * (k + 1))
                t3 = bufs[k].ap().rearrange("p (a q) -> p a q", a=2)
                nc.scalar.copy(smalls[:, 9 + 2 * j : 11 + 2 * j], t3[:, :, 0]).then_inc(
                    sed, 1
                )
                nc.scalar.copy(
                    smalls[:, 17 + 2 * j : 19 + 2 * j], t3[:, :, W - 1]
                ).then_inc(sed, 1)
                if j < 3:
                    nc.scalar.activation(
                        scratch[:], bufs[k][:], Copy,
                        accum_out=smalls[:, 4 + j : 5 + j],
                    ).then_inc(sed, 1)
                else:
                    # second half of the last y chunk (vector does the first)
                    nc.scalar.activation(
                        scratch[:, 0:W], bufs[k][:, W : 2 * W], Copy,
                        accum_out=smalls[:, 8:9],
                    ).then_inc(sed, 1)

    nc.compile()
    return nc


def _get_program():
    if "nc" not in _CACHE:
        _CACHE["nc"] = _build_program()
    return _CACHE["nc"]


def _tent(z):
    return np.maximum(0.0, 1.0 - np.abs(z))


def _warp_sum(sum_y, row0, row1, c0, c1, A):
    """sum(y_comp) from sum(y) + border strips, given phi_inv = A (f32)."""
    A64 = A.astype(np.float64)
    ap, bb = A64[0, 0] - 1.0, A64[0, 1]
    cc, dp = A64[1, 0], A64[1, 1] - 1.0
    e1, e2 = 1023.0 * A64[0, 2], 1023.0 * A64[1, 2]

    # sub-pixel displacement assumption |u|,|v| < 0.5 (checked at corners;
    # the fields are affine so corners bound the interior)
    mu = max(abs(ap * i + bb * j + e1) for i in (0.0, 1023.0) for j in (0.0, 1023.0))
    mv = max(abs(cc * i + dp * j + e2) for i in (0.0, 1023.0) for j in (0.0, 1023.0))
    assert mu < 0.5 and mv < 0.5, (mu, mv)

    kappa = (1.0 - ap) * (1.0 - dp) + bb * cc

    def g_true(p, q):
        g = np.zeros(np.broadcast(p, q).shape)
        for di in (-1, 0, 1):
            for dj in (-1, 0, 1):
                i_, j_ = p - di, q - dj
                valid = (i_ >= 0) & (i_ < H) & (j_ >= 0) & (j_ < W)
                z1 = ap * i_ + bb * j_ + e1 - di
                z2 = cc * i_ + dp * j_ + e2 - dj
                g += _tent(z1) * _tent(z2) * valid
        return g

    qs = np.arange(W, dtype=np.float64)
    ps = np.arange(1, H - 1, dtype=np.float64)
    ds = 0.0
    ds += np.sum(row0.astype(np.float64) * (g_true(0.0, qs) - kappa))
    ds += np.sum(row1.astype(np.float64) * (g_true(1023.0, qs) - kappa))
    ds += np.sum(c0[1:-1].astype(np.float64) * (g_true(ps, 0.0) - kappa))
    ds += np.sum(c1[1:-1].astype(np.float64) * (g_true(ps, 1023.0) - kappa))

    return kappa * float(sum_y) + ds


def _affine_f32(feat32, Wl, bl):
    M = (feat32 @ Wl + bl).reshape(3, 3)
    return np.eye(3, dtype=np.float32) + np.float32(0.01) * M


def kernel(x, y, Wpsi, bpsi, Wphi, bphi):
    from concourse import bass2jax

    B = x.shape[0]
    assert x.shape == (B, 1, H, W) and y.shape == (B, 1, H, W)

    nc = _get_program()
    in_maps = [
        {"x": np.ascontiguousarray(x[b, 0]), "y": np.ascontiguousarray(y[b, 0])}
        for b in range(B)
    ]
    results = bass2jax.run_bass_via_pjrt(nc, in_maps, n_cores=B)

    out = np.empty((B, 3, 3), dtype=np.float32)
    inv_hw = 1.0 / float(H * W)
    for b in range(B):
        r = np.asarray(results[b]["out"], dtype=np.float32).reshape(-1)
        sm = r[0:3200].reshape(128, 25).astype(np.float64)
        sum_x = float(sm[:, 0:4].sum())
        sum_y = float(sm[:, 4:9].sum())
        # strip cols land p-major: sm[p, 9+kblk] = y[kblk*128 + p, col]
        c0 = sm[:, 9:17].T.ravel()
        c1 = sm[:, 17:25].T.ravel()
        row0 = r[3200:4224].astype(np.float64)
        row1 = r[4224:5248].astype(np.float64)

        mean_x = np.float32(sum_x * inv_hw)
        mean_y = np.float32(sum_y * inv_hw)
        phi = _affine_f32(np.array([mean_x, mean_y], np.float32), Wpsi, bpsi)
        A = np.linalg.inv(phi)

        s_comp = _warp_sum(sum_y, row0, row1, c0, c1, A)
        mean_yc = np.float32(s_comp * inv_hw)

        psi = _affine_f32(np.array([mean_x, mean_yc], np.float32), Wphi, bphi)
        out[b] = phi + psi - np.eye(3, dtype=np.float32)
    return out
